# revision 13
# baseline (speedup 1.0000x reference)
"""Encoder-decoder attention (d_model=512, h=8 heads, d_k=d_v=64, S=2048),
head-parallel across 8 NeuronCores — one head per core.

Per-core Bass/Tile kernel, all-bf16 data path (fp32 only inside PSUM):
  qT[64,2048]  = Wq_h^T emb^T            (no bias — see below)
  kT[64,2048]  = Wk_h^T K^T              (no bias)
  cT[t]        = (bq/8)·k_raw[t]         (16 rank-1 matmuls, [128,1] tiles)
  v[2048,65]   = (V [Wv|0; bv 1])        (65th col = ones, for the denom)
  ST[t,s]      = kT-tile^T @ qT          (scores transposed; K=64)
  E = exp(ST/8 + cT)                     (ACT, scale+per-partition bias fused)
  psum_o[65,s] = [v|1]^T @ E             (accumulated over t; row 64 = denom)
  out_c        = num^T @ Wo_h            (UNNORMALIZED, DMA'd from PSUM)
  den_c        = denom row               (DMA'd from PSUM)
Host: out = sum_h out_h / den_h + bo.

Bias math: softmax over t of (q+bq)·(k+bk)/8 — the q·bk and bq·bk terms are
constant in t and cancel in the softmax, so only bq·k_raw[t] survives; it is
folded into the exp as a per-partition (t) bias. V's bias rides along as a
rank-1 update; its effect (+bv per output row) survives the softmax exactly.
"""

import numpy as np

import concourse.bass as bass
import concourse.mybir as mybir
import concourse.tile as tile
from concourse.bass_utils import run_bass_kernel_spmd

F32 = mybir.dt.float32
BF16 = mybir.dt.bfloat16
D_MODEL, H, DK = 512, 8, 64
S = 2048  # both S_q and S_kv
NT = S // 128  # 16 key tiles
NSC = S // 512  # 4 query chunks
N_CORES = 8
ND = D_MODEL // 128  # 4 contraction chunks


# The walrus build in this container rejects >1 sync-wait per instruction.
# Tile freely attaches several waits to one instruction (multi-producer
# deps, the kernel-tail drain), so after scheduling, move all but the last
# wait of each instruction onto same-engine NoOps inserted just before it —
# the sequencer blocks on each in turn, which is semantically identical.
def _split_multi_waits(nc):
    n_split = 0
    for fn in nc.m.functions:
        for bb in fn.blocks:
            out = []
            for inst in bb.instructions:
                si = inst.sync_info
                waits = list(si.on_wait) if (si is not None and si.on_wait) else []
                if len(waits) > 1:
                    for w in waits[:-1]:
                        n_split += 1
                        nop = mybir.InstNoOp(
                            name=f"I-wsplit-{n_split}", ins=[], outs=[]
                        )
                        nop.engine = inst.engine
                        nop.sync_info = mybir.SyncInfo(on_wait=[w], on_update=[])
                        nc.register_instruction(nop, overwrite=True)
                        out.append(nop)
                    si.on_wait = [waits[-1]]
                out.append(inst)
            bb.instructions = out


def build_program(reps=1, ablate=()):
    """Build the per-core Bass program (same program on all 8 cores).
    reps>1 repeats the compute body in-NEFF (for device-time measurement:
    the R8-vs-R1 wall-clock difference cancels dispatch overhead)."""
    nc = bass.Bass("TRN2", target_bir_lowering=False, debug=False)

    embT = nc.dram_tensor("embT", [D_MODEL, S], BF16, kind="ExternalInput").ap()
    kT_in = nc.dram_tensor("kT_in", [D_MODEL, S], BF16, kind="ExternalInput").ap()
    vT_in = nc.dram_tensor("vT_in", [D_MODEL, S], BF16, kind="ExternalInput").ap()
    wq_in = nc.dram_tensor("wq", [D_MODEL, DK], BF16, kind="ExternalInput").ap()
    wk_in = nc.dram_tensor("wk", [D_MODEL, DK], BF16, kind="ExternalInput").ap()
    bq8_in = nc.dram_tensor("bq8", [DK, 1], BF16, kind="ExternalInput").ap()
    wv_in = nc.dram_tensor("wv", [D_MODEL + 1, DK + 1], BF16, kind="ExternalInput").ap()
    wo_in = nc.dram_tensor("wo", [DK, D_MODEL], BF16, kind="ExternalInput").ap()
    out = nc.dram_tensor("out", [S, D_MODEL], F32, kind="ExternalOutput").ap()
    den = nc.dram_tensor("den", [1, S], F32, kind="ExternalOutput").ap()

    with tile.TileContext(nc) as tc:
        with (
            tc.tile_pool(name="io", bufs=1) as iop,
            tc.tile_pool(name="wp", bufs=1) as wp,
            tc.tile_pool(name="cst", bufs=1) as cst,
            tc.tile_pool(name="qk", bufs=2) as qkp,
            tc.tile_pool(name="vp", bufs=2) as vp,
            tc.tile_pool(name="ct", bufs=2) as ctp,
            tc.tile_pool(name="ep", bufs=6) as ep,
            tc.tile_pool(name="nump", bufs=2) as nump,
            tc.tile_pool(name="outp", bufs=4) as outp,
            tc.tile_pool(name="denp", bufs=2) as denp,
            tc.tile_pool(name="pacc", bufs=2, space="PSUM") as pacc,
            tc.tile_pool(name="ps", bufs=2, space="PSUM") as psp,
            tc.tile_pool(name="po", bufs=1, space="PSUM") as pop,
        ):
            # constants
            ones_row = cst.tile([1, S], BF16, tag="ones_row")
            nc.vector.memset(ones_row[:], 1.0)

            # resident input tiles
            emb_t, k_t, v_t = [], [], []
            for lst, dram, nm in (
                (emb_t, embT, "e"),
                (k_t, kT_in, "k"),
                (v_t, vT_in, "v"),
            ):
                for d in range(ND):
                    sl = slice(d * 128, (d + 1) * 128)
                    t = iop.tile([128, S], BF16, tag=f"{nm}{d}")
                    nc.sync.dma_start(t[:], dram[sl, :])
                    lst.append(t)

            # weights
            def load_w(dram, nm):
                chunks = []
                for d in range(ND):
                    t = wp.tile([128, dram.shape[1]], BF16, tag=f"{nm}{d}")
                    nc.sync.dma_start(t[:], dram[d * 128 : (d + 1) * 128, :])
                    chunks.append(t)
                return chunks

            wq_t = load_w(wq_in, "wq")
            wk_t = load_w(wk_in, "wk")
            wv_t = load_w(wv_in, "wv")
            wv_b = wp.tile([1, DK + 1], BF16, tag="wvb")
            nc.sync.dma_start(wv_b[:], wv_in[D_MODEL : D_MODEL + 1, :])
            bq8_sb = wp.tile([DK, 1], BF16, tag="bq8")
            nc.sync.dma_start(bq8_sb[:], bq8_in[:, :])
            wo_sb = wp.tile([DK, D_MODEL], BF16, tag="wo")
            nc.sync.dma_start(wo_sb[:], wo_in[:, :])

            # ───────────────────────────────────────────────────────
            # Emission helpers. The whole kernel is one software pipeline:
            # rep r's ACT-paced attention loop drains a background queue
            # holding rep r+1's projections and rep r's deferred epilogue
            # matmuls, so the strict-FIFO PE never idles between phases.
            # ───────────────────────────────────────────────────────
            n_it = (NSC // 2) * NT

            def emit_proj_chunk(dst, wt, src, sc):
                ssl = slice(sc * 512, (sc + 1) * 512)
                ps = pacc.tile([DK, 512], F32, tag="acc", name="ps")
                for d in range(ND):
                    nc.tensor.matmul(
                        ps[:], wt[d][:], src[d][:, ssl],
                        start=(d == 0), stop=(d == ND - 1),
                    )
                nc.vector.tensor_copy(dst[0:DK, ssl], ps[:])
                # row-group duplicate via DMA (SBUF→SBUF) to keep DVE free
                nc.sync.dma_start(dst[DK : 2 * DK, ssl], dst[0:DK, ssl])

            def emit_v_quad(vq, tq):
                # four t-tiles share one psum bank (4×65=260 f32 columns) so
                # the pacc slot rotation isn't paced by tiny copies
                pv = pacc.tile([128, 4 * (DK + 1)], F32, tag="acc", name="pv")
                for i in range(4):
                    t = 4 * tq + i
                    tsl = slice(t * 128, (t + 1) * 128)
                    csl = slice(i * (DK + 1), (i + 1) * (DK + 1))
                    for d in range(ND):
                        nc.tensor.matmul(
                            pv[:, csl], v_t[d][:, tsl], wv_t[d][:],
                            start=(d == 0), stop=False,
                            skip_group_check=True,
                        )
                    nc.tensor.matmul(
                        pv[:, csl], ones_row[:, tsl], wv_b[:],
                        start=False, stop=True, skip_group_check=True,
                    )
                nc.vector.tensor_copy(vq[:], pv[:])

            def emit_ct(cT_all, kT):
                # exp-bias columns (bq/8)·k_raw[t]: 16 rank-1 matmuls into
                # ONE psum bank, single eviction
                pcT = pacc.tile([128, NT], F32, tag="acc", name="pcT")
                for t in range(NT):
                    tsl = slice(t * 128, (t + 1) * 128)
                    nc.tensor.matmul(
                        pcT[:, t : t + 1], kT[0:DK, tsl], bq8_sb[:],
                        start=True, stop=True, skip_group_check=True,
                    )
                nc.vector.tensor_copy(cT_all[:], pcT[:])

            def make_proj(r):
                """Create rep r's tiles; return (closures, handles)."""
                # [128, S]: rows 0:64 and 64:128 hold the SAME data, so
                # odd key-tiles' score matmuls can sit in PE row-group (64,0)
                # and run concurrently with even tiles' in (0,0)
                qT = qkp.tile([128, S], BF16, tag="qT", name="qT")
                kT = qkp.tile([128, S], BF16, tag="kT", name="kT")
                vqs = [
                    vp.tile([128, 4 * (DK + 1)], BF16, tag=f"v{tq}", name="vq")
                    for tq in range(NT // 4)
                ]
                cT_all = ctp.tile([128, NT], F32, tag="cT", name="cT_all")
                cl = []
                for sc in range(NSC):
                    cl.append(lambda sc=sc: emit_proj_chunk(qT, wq_t, emb_t, sc))
                for sc in range(NSC):
                    cl.append(lambda sc=sc, kT=kT: emit_proj_chunk(kT, wk_t, k_t, sc))
                for tq in range(NT // 4):
                    cl.append(lambda tq=tq, vq=vqs[tq]: emit_v_quad(vq, tq))
                cl.append(lambda cT_all=cT_all, kT=kT: emit_ct(cT_all, kT))
                H = dict(
                    qT=qT,
                    kT=kT,
                    v_sb=[
                        vqs[t // 4][:, (t % 4) * (DK + 1) : (t % 4 + 1) * (DK + 1)]
                        for t in range(NT)
                    ],
                    cT_sb=[cT_all[:, t : t + 1] for t in range(NT)],
                )
                return cl, H

            def epi_mm_closures(numT_sb, scp):
                """Deferred epilogue: per (sc, j) out-proj matmul + eviction
                (alternating Pool/DVE) + store. One closure per j."""
                cl = []

                def one(sc, j):
                    jsl = slice(j * 128, (j + 1) * 128)
                    ppj = pacc.tile([128, 512], F32, tag="acc", name="ppj")
                    nc.tensor.matmul(
                        ppj[:], numT_sb[sc][:, jsl], wo_sb[:],
                        start=True, stop=True,
                    )
                    ob = outp.tile([128, 512], F32, tag="ob", name="ob")
                    nc.vector.tensor_copy(ob[:], ppj[:])
                    nc.sync.dma_start(
                        out[sc * 512 + j * 128 : sc * 512 + (j + 1) * 128, :],
                        ob[:],
                    )

                for sc in (2 * scp, 2 * scp + 1):
                    for j in range(4):
                        cl.append(lambda sc=sc, j=j: one(sc, j))
                return cl

            def emit_attention(H, bg, on_scp0_done):
                """One rep's attention: 32 pipelined iterations draining one
                background closure each. scores(t+2) issues before attnV(t)
                so the FIFO PE never sits behind exp(t)."""
                qT, kT = H["qT"], H["kT"]
                v_sb, cT_sb = H["v_sb"], H["cT_sb"]
                po_t, ex_q, numT_sb = {}, [], {}

                def sc_emit(it):
                    scp, t = divmod(it, NT)
                    tsl = slice(t * 128, (t + 1) * 128)
                    psl = slice(0, DK) if t % 2 == 0 else slice(DK, 2 * DK)
                    pscore = psp.tile([128, 1024], F32, tag="s", name="pscore")
                    for half, sc in enumerate((2 * scp, 2 * scp + 1)):
                        ssl = slice(sc * 512, (sc + 1) * 512)
                        nc.tensor.matmul(
                            pscore[:, half * 512 : (half + 1) * 512],
                            kT[psl, tsl], qT[psl, ssl],
                            start=True, stop=True, skip_group_check=True,
                        )
                    ex = ep.tile([128, 1024], BF16, tag="ex", name="ex")
                    nc.scalar.activation(
                        ex[:], pscore[:], mybir.ActivationFunctionType.Exp,
                        scale=0.125, bias=cT_sb[t][:],
                    )
                    ex_q.append(ex)

                def av_emit(it):
                    scp, t = divmod(it, NT)
                    ex = ex_q.pop(0)
                    for half, sc in enumerate((2 * scp, 2 * scp + 1)):
                        if t == 0:
                            po_t[sc] = pop.tile(
                                [DK + 1, 512], F32, tag=f"o{half}",
                                name=f"po{half}",
                            )
                        nc.tensor.matmul(
                            po_t[sc][:], v_sb[t][:],
                            ex[:, half * 512 : (half + 1) * 512],
                            start=(t == 0), stop=(t == NT - 1),
                            skip_group_check=True,
                        )

                def epi_copies(scp):
                    for sc in (2 * scp, 2 * scp + 1):
                        numT = nump.tile(
                            [DK, 512], BF16, tag=f"numT{sc % 2}", name="numT"
                        )
                        nc.vector.tensor_copy(numT[:], po_t[sc][0:DK, :])
                        dn = denp.tile([1, 512], F32, tag="dn", name="dn")
                        nc.vector.tensor_copy(dn[:], po_t[sc][DK : DK + 1, :])
                        nc.sync.dma_start(
                            den[0:1, sc * 512 : (sc + 1) * 512], dn[:]
                        )
                        numT_sb[sc] = numT

                sc_emit(0)
                sc_emit(1)
                for it in range(n_it):
                    if it % 2 == 0:
                        if it + 2 < n_it:
                            sc_emit(it + 2)
                        if it + 3 < n_it:
                            sc_emit(it + 3)
                    av_emit(it)
                    scp, t = divmod(it, NT)
                    if t == NT - 1:
                        epi_copies(scp)
                        if scp == 0:
                            on_scp0_done(numT_sb, bg)
                    if bg:
                        bg.pop(0)()
                return numT_sb

            # ───────────────────────────────────────────────────────
            # main pipeline over reps
            # ───────────────────────────────────────────────────────
            cur_cl, cur_H = make_proj(0)
            for c in cur_cl:
                c()
            pend_epi = []  # previous rep's scp-1 epilogue closures
            for r in range(reps):
                nxt = make_proj(r + 1) if r + 1 < reps else None
                bg = list(pend_epi)
                if nxt is not None:
                    bg.extend(nxt[0])

                def on_scp0_done(numT_sb, bg):
                    bg.extend(epi_mm_closures(numT_sb, 0))

                numT_sb = emit_attention(cur_H, bg, on_scp0_done)
                for c in bg:  # anything not drained (safety)
                    c()
                pend_epi = epi_mm_closures(numT_sb, 1)
                if nxt is not None:
                    cur_H = nxt[1]
            for c in pend_epi:
                c()

    _split_multi_waits(nc)
    return nc


_NC = None


def _get_nc():
    global _NC
    if _NC is None:
        _NC = build_program()
    return _NC


def make_in_maps(inputs):
    """Host-side shard: transpose + bf16-cast the shared activations once,
    slice per-head weights, fold V's bias in as an extra weight row."""
    bf16 = mybir.dt.np(mybir.dt.bfloat16)
    emb = np.asarray(inputs["embeddings"], np.float32)
    K = np.asarray(inputs["K"], np.float32)
    V = np.asarray(inputs["V"], np.float32)
    Wq = np.asarray(inputs["Wq"], np.float32)
    bq = np.asarray(inputs["bq"], np.float32)
    Wk = np.asarray(inputs["Wk"], np.float32)
    Wv = np.asarray(inputs["Wv"], np.float32)
    bv = np.asarray(inputs["bv"], np.float32)
    Wo = np.asarray(inputs["Wo"], np.float32)

    embT = np.ascontiguousarray(emb.T).astype(bf16)
    kT = np.ascontiguousarray(K.T).astype(bf16)
    vT = np.ascontiguousarray(V.T).astype(bf16)

    in_maps = []
    for h in range(N_CORES):
        wv_aug = np.concatenate(
            [
                np.concatenate([Wv[h], bv[h][None, :]], axis=0),
                np.concatenate(
                    [np.zeros((D_MODEL, 1), np.float32),
                     np.ones((1, 1), np.float32)], axis=0
                ),
            ],
            axis=1,
        )
        in_maps.append(
            {
                "embT": embT,
                "kT_in": kT,
                "vT_in": vT,
                "wq": np.ascontiguousarray(Wq[h]).astype(bf16),
                "wk": np.ascontiguousarray(Wk[h]).astype(bf16),
                "bq8": np.ascontiguousarray((bq[h] / 8.0)[:, None]).astype(bf16),
                "wv": np.ascontiguousarray(wv_aug).astype(bf16),
                "wo": np.ascontiguousarray(Wo[h * DK : (h + 1) * DK, :]).astype(bf16),
            }
        )
    return in_maps


def kernel(**inputs):
    nc = _get_nc()
    in_maps = make_in_maps(inputs)
    res = run_bass_kernel_spmd(nc, in_maps, core_ids=list(range(N_CORES)))
    bo = np.asarray(inputs["bo"], np.float32)
    acc = None
    for c in range(N_CORES):
        o = np.asarray(res.results[c]["out"], np.float32)
        d = np.asarray(res.results[c]["den"], np.float32).reshape(S, 1)
        part = o / d
        acc = part if acc is None else acc + part
    return (acc + bo[None, :]).astype(np.float32)


# revision 14
# speedup vs baseline: 2.9653x; 2.9653x over previous
"""Encoder-decoder attention (d_model=512, h=8 heads, d_k=d_v=64, S=2048),
head-parallel across 8 NeuronCores — one head per core.

Per-core Bass/Tile kernel, all-bf16 data path (fp32 only inside PSUM):
  qT[64,2048]  = Wq_h^T emb^T            (no bias — see below)
  kT[64,2048]  = Wk_h^T K^T              (no bias)
  cT[t]        = (bq/8)·k_raw[t]         (16 rank-1 matmuls, [128,1] tiles)
  v[2048,65]   = (V [Wv|0; bv 1])        (65th col = ones, for the denom)
  ST[t,s]      = kT-tile^T @ qT          (scores transposed; K=64)
  E = exp(ST/8 + cT)                     (ACT, scale+per-partition bias fused)
  psum_o[65,s] = [v|1]^T @ E             (accumulated over t; row 64 = denom)
  out_c        = num^T @ Wo_h            (UNNORMALIZED, DMA'd from PSUM)
  den_c        = denom row               (DMA'd from PSUM)
Host: out = sum_h out_h / den_h + bo.

Bias math: softmax over t of (q+bq)·(k+bk)/8 — the q·bk and bq·bk terms are
constant in t and cancel in the softmax, so only bq·k_raw[t] survives; it is
folded into the exp as a per-partition (t) bias. V's bias rides along as a
rank-1 update; its effect (+bv per output row) survives the softmax exactly.
"""

import numpy as np

import concourse.bass as bass
import concourse.mybir as mybir
import concourse.tile as tile
from concourse.bass_utils import run_bass_kernel_spmd

F32 = mybir.dt.float32
BF16 = mybir.dt.bfloat16
D_MODEL, H, DK = 512, 8, 64
S = 2048  # both S_q and S_kv
NT = S // 128  # 16 key tiles
NSC = S // 512  # 4 query chunks
N_CORES = 8
ND = D_MODEL // 128  # 4 contraction chunks


# The walrus build in this container rejects >1 sync-wait per instruction.
# Tile freely attaches several waits to one instruction (multi-producer
# deps, the kernel-tail drain), so after scheduling, move all but the last
# wait of each instruction onto same-engine NoOps inserted just before it —
# the sequencer blocks on each in turn, which is semantically identical.
def _split_multi_waits(nc):
    n_split = 0
    for fn in nc.m.functions:
        for bb in fn.blocks:
            out = []
            for inst in bb.instructions:
                si = inst.sync_info
                waits = list(si.on_wait) if (si is not None and si.on_wait) else []
                if len(waits) > 1:
                    for w in waits[:-1]:
                        n_split += 1
                        nop = mybir.InstNoOp(
                            name=f"I-wsplit-{n_split}", ins=[], outs=[]
                        )
                        nop.engine = inst.engine
                        nop.sync_info = mybir.SyncInfo(on_wait=[w], on_update=[])
                        nc.register_instruction(nop, overwrite=True)
                        out.append(nop)
                    si.on_wait = [waits[-1]]
                out.append(inst)
            bb.instructions = out


def build_program(reps=1, ablate=()):
    """Build the per-core Bass program (same program on all 8 cores).
    reps>1 repeats the compute body in-NEFF (for device-time measurement:
    the R8-vs-R1 wall-clock difference cancels dispatch overhead)."""
    nc = bass.Bass("TRN2", target_bir_lowering=False, debug=False)

    embT = nc.dram_tensor("embT", [D_MODEL, S], BF16, kind="ExternalInput").ap()
    kT_in = nc.dram_tensor("kT_in", [D_MODEL, S], BF16, kind="ExternalInput").ap()
    vT_in = nc.dram_tensor("vT_in", [D_MODEL, S], BF16, kind="ExternalInput").ap()
    wq_in = nc.dram_tensor("wq", [D_MODEL, DK], BF16, kind="ExternalInput").ap()
    wk_in = nc.dram_tensor("wk", [D_MODEL, DK], BF16, kind="ExternalInput").ap()
    bq8_in = nc.dram_tensor("bq8", [DK, 1], BF16, kind="ExternalInput").ap()
    wv_in = nc.dram_tensor("wv", [D_MODEL + 1, DK + 1], BF16, kind="ExternalInput").ap()
    wo_in = nc.dram_tensor("wo", [DK, D_MODEL], BF16, kind="ExternalInput").ap()
    out = nc.dram_tensor("out", [S, D_MODEL], F32, kind="ExternalOutput").ap()
    den = nc.dram_tensor("den", [1, S], F32, kind="ExternalOutput").ap()

    with tile.TileContext(nc) as tc:
        with (
            tc.tile_pool(name="io", bufs=1) as iop,
            tc.tile_pool(name="wp", bufs=1) as wp,
            tc.tile_pool(name="cst", bufs=1) as cst,
            tc.tile_pool(name="qk", bufs=2) as qkp,
            tc.tile_pool(name="vp", bufs=2) as vp,
            tc.tile_pool(name="ct", bufs=2) as ctp,
            tc.tile_pool(name="ep", bufs=6) as ep,
            tc.tile_pool(name="nump", bufs=2) as nump,
            tc.tile_pool(name="outp", bufs=4) as outp,
            tc.tile_pool(name="denp", bufs=2) as denp,
            tc.tile_pool(name="pacc", bufs=2, space="PSUM") as pacc,
            tc.tile_pool(name="ps", bufs=2, space="PSUM") as psp,
            tc.tile_pool(name="po", bufs=1, space="PSUM") as pop,
        ):
            # constants
            ones_row = cst.tile([1, S], BF16, tag="ones_row")
            nc.vector.memset(ones_row[:], 1.0)

            # weights
            def load_w(dram, nm):
                chunks = []
                for d in range(ND):
                    t = wp.tile([128, dram.shape[1]], BF16, tag=f"{nm}{d}")
                    nc.sync.dma_start(t[:], dram[d * 128 : (d + 1) * 128, :])
                    chunks.append(t)
                return chunks

            wq_t = load_w(wq_in, "wq")
            wk_t = load_w(wk_in, "wk")
            wv_t = load_w(wv_in, "wv")
            wv_b = wp.tile([1, DK + 1], BF16, tag="wvb")
            nc.sync.dma_start(wv_b[:], wv_in[D_MODEL : D_MODEL + 1, :])
            bq8_sb = wp.tile([DK, 1], BF16, tag="bq8")
            nc.sync.dma_start(bq8_sb[:], bq8_in[:, :])
            wo_sb = wp.tile([DK, D_MODEL], BF16, tag="wo")
            nc.sync.dma_start(wo_sb[:], wo_in[:, :])

            # resident input tiles
            emb_t, k_t, v_t = [], [], []
            for lst, dram, nm in (
                (emb_t, embT, "e"),
                (k_t, kT_in, "k"),
                (v_t, vT_in, "v"),
            ):
                for d in range(ND):
                    sl = slice(d * 128, (d + 1) * 128)
                    t = iop.tile([128, S], BF16, tag=f"{nm}{d}")
                    nc.sync.dma_start(t[:], dram[sl, :])
                    lst.append(t)

            # ───────────────────────────────────────────────────────
            # Emission helpers. The whole kernel is one software pipeline:
            # rep r's ACT-paced attention loop drains a background queue
            # holding rep r+1's projections and rep r's deferred epilogue
            # matmuls, so the strict-FIFO PE never idles between phases.
            # ───────────────────────────────────────────────────────
            n_it = (NSC // 2) * NT

            def emit_proj_chunk(dst, wt, src, sc):
                ssl = slice(sc * 512, (sc + 1) * 512)
                ps = pacc.tile([DK, 512], F32, tag="acc", name="ps")
                for d in range(ND):
                    nc.tensor.matmul(
                        ps[:], wt[d][:], src[d][:, ssl],
                        start=(d == 0), stop=(d == ND - 1),
                    )
                nc.vector.tensor_copy(dst[0:DK, ssl], ps[:])
                # row-group duplicate via DMA (SBUF→SBUF) to keep DVE free
                nc.sync.dma_start(dst[DK : 2 * DK, ssl], dst[0:DK, ssl])

            def emit_v_quad(vq, tq):
                # four t-tiles share one psum bank (4×65=260 f32 columns) so
                # the pacc slot rotation isn't paced by tiny copies
                pv = pacc.tile([128, 4 * (DK + 1)], F32, tag="acc", name="pv")
                for i in range(4):
                    t = 4 * tq + i
                    tsl = slice(t * 128, (t + 1) * 128)
                    csl = slice(i * (DK + 1), (i + 1) * (DK + 1))
                    for d in range(ND):
                        nc.tensor.matmul(
                            pv[:, csl], v_t[d][:, tsl], wv_t[d][:],
                            start=(d == 0), stop=False,
                            skip_group_check=True,
                        )
                    nc.tensor.matmul(
                        pv[:, csl], ones_row[:, tsl], wv_b[:],
                        start=False, stop=True, skip_group_check=True,
                    )
                nc.vector.tensor_copy(vq[:], pv[:])

            def emit_ct(cT_all, kT):
                # exp-bias columns (bq/8)·k_raw[t]: 16 rank-1 matmuls into
                # ONE psum bank, single eviction
                pcT = pacc.tile([128, NT], F32, tag="acc", name="pcT")
                for sc in range(NSC):
                    for t in range(sc * 4, (sc + 1) * 4):
                        tsl = slice(t * 128, (t + 1) * 128)
                        nc.tensor.matmul(
                            pcT[:, t : t + 1], kT[0:DK, tsl], bq8_sb[:],
                            start=True, stop=True, skip_group_check=True,
                        )
                    nc.vector.tensor_copy(
                        cT_all[:, sc * 4 : (sc + 1) * 4],
                        pcT[:, sc * 4 : (sc + 1) * 4],
                    )

            def make_proj(r):
                """Create rep r's tiles; return (closures, handles)."""
                # [128, S]: rows 0:64 and 64:128 hold the SAME data, so
                # odd key-tiles' score matmuls can sit in PE row-group (64,0)
                # and run concurrently with even tiles' in (0,0)
                qT = qkp.tile([128, S], BF16, tag="qT", name="qT")
                kT = qkp.tile([128, S], BF16, tag="kT", name="kT")
                vqs = [
                    vp.tile([128, 4 * (DK + 1)], BF16, tag=f"v{tq}", name="vq")
                    for tq in range(NT // 4)
                ]
                cT_all = ctp.tile([128, NT], F32, tag="cT", name="cT_all")
                cl = []
                for sc in range(NSC):
                    cl.append(lambda sc=sc: emit_proj_chunk(qT, wq_t, emb_t, sc))
                for sc in range(NSC):
                    cl.append(lambda sc=sc, kT=kT: emit_proj_chunk(kT, wk_t, k_t, sc))
                for tq in range(NT // 4):
                    cl.append(lambda tq=tq, vq=vqs[tq]: emit_v_quad(vq, tq))
                cl.append(lambda cT_all=cT_all, kT=kT: emit_ct(cT_all, kT))
                H = dict(
                    qT=qT,
                    kT=kT,
                    v_sb=[
                        vqs[t // 4][:, (t % 4) * (DK + 1) : (t % 4 + 1) * (DK + 1)]
                        for t in range(NT)
                    ],
                    cT_sb=[cT_all[:, t : t + 1] for t in range(NT)],
                )
                return cl, H

            def epi_mm_closures(numT_sb, scp, tail=False):
                """Deferred epilogue: per (sc, j) out-proj matmul + eviction
                + store. One closure per j. In the tail (nothing left for
                ACT to do) half the evictions go through the scalar engine
                so the pacc rotation isn't DVE-chained."""
                cl = []

                def one(sc, j):
                    jsl = slice(j * 128, (j + 1) * 128)
                    ppj = pacc.tile([128, 512], F32, tag="acc", name="ppj")
                    nc.tensor.matmul(
                        ppj[:], numT_sb[sc][:, jsl], wo_sb[:],
                        start=True, stop=True,
                    )
                    ob = outp.tile([128, 512], F32, tag="ob", name="ob")
                    if tail and j % 2 == 0:
                        nc.scalar.copy(ob[:], ppj[:])
                    else:
                        nc.vector.tensor_copy(ob[:], ppj[:])
                    nc.sync.dma_start(
                        out[sc * 512 + j * 128 : sc * 512 + (j + 1) * 128, :],
                        ob[:],
                    )

                for sc in (2 * scp, 2 * scp + 1):
                    for j in range(4):
                        cl.append(lambda sc=sc, j=j: one(sc, j))
                return cl

            def emit_attention(H, bg, on_scp0_done):
                """One rep's attention: 32 pipelined iterations draining one
                background closure each. scores(t+2) issues before attnV(t)
                so the FIFO PE never sits behind exp(t)."""
                qT, kT = H["qT"], H["kT"]
                v_sb, cT_sb = H["v_sb"], H["cT_sb"]
                po_t, ex_q, numT_sb = {}, [], {}

                def sc_emit(it):
                    scp, t = divmod(it, NT)
                    tsl = slice(t * 128, (t + 1) * 128)
                    psl = slice(0, DK) if t % 2 == 0 else slice(DK, 2 * DK)
                    pscore = psp.tile([128, 1024], F32, tag="s", name="pscore")
                    for half, sc in enumerate((2 * scp, 2 * scp + 1)):
                        ssl = slice(sc * 512, (sc + 1) * 512)
                        nc.tensor.matmul(
                            pscore[:, half * 512 : (half + 1) * 512],
                            kT[psl, tsl], qT[psl, ssl],
                            start=True, stop=True, skip_group_check=True,
                        )
                    ex = ep.tile([128, 1024], BF16, tag="ex", name="ex")
                    nc.scalar.activation(
                        ex[:], pscore[:], mybir.ActivationFunctionType.Exp,
                        scale=0.125, bias=cT_sb[t][:],
                    )
                    ex_q.append(ex)

                def av_emit(it):
                    scp, t = divmod(it, NT)
                    ex = ex_q.pop(0)
                    for half, sc in enumerate((2 * scp, 2 * scp + 1)):
                        if t == 0:
                            po_t[sc] = pop.tile(
                                [DK + 1, 512], F32, tag=f"o{half}",
                                name=f"po{half}",
                            )
                        nc.tensor.matmul(
                            po_t[sc][:], v_sb[t][:],
                            ex[:, half * 512 : (half + 1) * 512],
                            start=(t == 0), stop=(t == NT - 1),
                            skip_group_check=True,
                        )

                def epi_copies(scp):
                    for sc in (2 * scp, 2 * scp + 1):
                        numT = nump.tile(
                            [DK, 512], BF16, tag=f"numT{sc % 2}", name="numT"
                        )
                        nc.vector.tensor_copy(numT[:], po_t[sc][0:DK, :])
                        dn = denp.tile([1, 512], F32, tag="dn", name="dn")
                        nc.vector.tensor_copy(dn[:], po_t[sc][DK : DK + 1, :])
                        nc.sync.dma_start(
                            den[0:1, sc * 512 : (sc + 1) * 512], dn[:]
                        )
                        numT_sb[sc] = numT

                sc_emit(0)
                sc_emit(1)
                for it in range(n_it):
                    if it % 2 == 0:
                        if it + 2 < n_it:
                            sc_emit(it + 2)
                        if it + 3 < n_it:
                            sc_emit(it + 3)
                    av_emit(it)
                    scp, t = divmod(it, NT)
                    if t == NT - 1:
                        epi_copies(scp)
                        if scp == 0:
                            on_scp0_done(numT_sb, bg)
                    if bg:
                        bg.pop(0)()
                return numT_sb

            # ───────────────────────────────────────────────────────
            # main pipeline over reps
            # ───────────────────────────────────────────────────────
            cur_cl, cur_H = make_proj(0)
            for c in cur_cl:
                c()
            pend_epi = []  # previous rep's scp-1 epilogue closures
            for r in range(reps):
                nxt = make_proj(r + 1) if r + 1 < reps else None
                bg = list(pend_epi)
                if nxt is not None:
                    bg.extend(nxt[0])

                def on_scp0_done(numT_sb, bg):
                    bg.extend(epi_mm_closures(numT_sb, 0))

                numT_sb = emit_attention(cur_H, bg, on_scp0_done)
                for c in bg:  # anything not drained (safety)
                    c()
                pend_epi = epi_mm_closures(numT_sb, 1, tail=(r == reps - 1))
                if nxt is not None:
                    cur_H = nxt[1]
            for c in pend_epi:
                c()

    _split_multi_waits(nc)
    return nc


_NC = None


def _get_nc():
    global _NC
    if _NC is None:
        _NC = build_program()
    return _NC


def make_in_maps(inputs):
    """Host-side shard: transpose + bf16-cast the shared activations once,
    slice per-head weights, fold V's bias in as an extra weight row."""
    bf16 = mybir.dt.np(mybir.dt.bfloat16)
    emb = np.asarray(inputs["embeddings"], np.float32)
    K = np.asarray(inputs["K"], np.float32)
    V = np.asarray(inputs["V"], np.float32)
    Wq = np.asarray(inputs["Wq"], np.float32)
    bq = np.asarray(inputs["bq"], np.float32)
    Wk = np.asarray(inputs["Wk"], np.float32)
    Wv = np.asarray(inputs["Wv"], np.float32)
    bv = np.asarray(inputs["bv"], np.float32)
    Wo = np.asarray(inputs["Wo"], np.float32)

    embT = np.ascontiguousarray(emb.T).astype(bf16)
    kT = np.ascontiguousarray(K.T).astype(bf16)
    vT = np.ascontiguousarray(V.T).astype(bf16)

    in_maps = []
    for h in range(N_CORES):
        wv_aug = np.concatenate(
            [
                np.concatenate([Wv[h], bv[h][None, :]], axis=0),
                np.concatenate(
                    [np.zeros((D_MODEL, 1), np.float32),
                     np.ones((1, 1), np.float32)], axis=0
                ),
            ],
            axis=1,
        )
        in_maps.append(
            {
                "embT": embT,
                "kT_in": kT,
                "vT_in": vT,
                "wq": np.ascontiguousarray(Wq[h]).astype(bf16),
                "wk": np.ascontiguousarray(Wk[h]).astype(bf16),
                "bq8": np.ascontiguousarray((bq[h] / 8.0)[:, None]).astype(bf16),
                "wv": np.ascontiguousarray(wv_aug).astype(bf16),
                "wo": np.ascontiguousarray(Wo[h * DK : (h + 1) * DK, :]).astype(bf16),
            }
        )
    return in_maps


def kernel(**inputs):
    nc = _get_nc()
    in_maps = make_in_maps(inputs)
    res = run_bass_kernel_spmd(nc, in_maps, core_ids=list(range(N_CORES)))
    bo = np.asarray(inputs["bo"], np.float32)
    acc = None
    for c in range(N_CORES):
        o = np.asarray(res.results[c]["out"], np.float32)
        d = np.asarray(res.results[c]["den"], np.float32).reshape(S, 1)
        part = o / d
        acc = part if acc is None else acc + part
    return (acc + bo[None, :]).astype(np.float32)
